# revision 13
# baseline (speedup 1.0000x reference)
"""KNN classify kernel for TRN2 (8 NeuronCores).

Strategy: shard X over N (12500 points/core, padded to 12800). Scores are
computed as s[b,n] = 2*q.x with two fp8e4 DoubleRow matmuls (K_eff=256 each
over the 512 dims). The -||x||^2 term is handled structurally: the host
sorts each core's points by ||x||^2 and permutes columns so that each
pooling window (16 columns congruent mod 128 within a 2048-wide slab) holds
16 norm-consecutive points, dealt round-robin across slabs. PSUM tiles are
drained to SBUF fp16 (Scalar engine mostly, DVE every 8th), folded by a DVE
pairwise-max tree to 128 window maxima per slab, then a per-window constant
c[w] = min ||x||^2 (an fp16 input) is subtracted so window values
approximate max(2qx - x^2) with error bounded by the within-window norm
spread. MAX8/FIND_INDEX8 extract the top-8 windows per slab-pair; the host
expands the top windows (16 columns each), rescores exactly in fp64, takes
top-K and emits label votes.
"""

import sys

sys.path.insert(0, "/opt/trn_rl_repo")

import ml_dtypes
import numpy as np

import concourse.bacc as bacc
import concourse.mybir as mybir
from concourse import bass_utils
from concourse.tile import TileContext

B, D, N = 2048, 512, 100000
NCORES = 8
NSH = N // NCORES  # 12500
NPAD = 12800  # 25 * 512
P = 128
BLK = B // P  # 16 query blocks
NF = 512  # psum bank width (fp32 cols)
SLAB = 2048  # 4 psum banks
NFULL = 6  # full slabs; + 1 partial slab of 512
WPART = NPAD - NFULL * SLAB  # 512
NSLABS = NFULL + 1
RW = 16  # columns per pooled window
NWIN = SLAB // RW  # 128 windows per full slab
NWINP = WPART // RW  # 32 windows in partial slab
NFWIN = NFULL * NWIN  # 768 windows live in full slabs; the rest go to the partial
NPAIR = NFULL // 2  # slab pairs sharing one top-8 extraction
NW = (NPAIR + 1) * 8  # 32 candidates per (core, query)
GB = 4  # query-block group size for batched folds
DVE_DRAIN_EVERY = 8  # every Nth slab-drain goes to DVE instead of Scalar

E4 = ml_dtypes.float8_e4m3
TOPW = 40  # windows expanded+rescored on host per query

_prog = None


def _build_program():
    nc = bacc.Bacc("TRN2", target_bir_lowering=False, debug=False, num_devices=NCORES)
    qt_d = nc.dram_tensor("qt", (BLK, P, 2, 2, P), mybir.dt.float8e4, kind="ExternalInput")
    xt_d = nc.dram_tensor("xt", (2, P, 2, NPAD), mybir.dt.float8e4, kind="ExternalInput")
    c_d = nc.dram_tensor("cwin", (P, NSLABS, GB, NWIN), mybir.dt.float16, kind="ExternalInput")
    vals_d = nc.dram_tensor("cand_vals", (B, NW), mybir.dt.float16, kind="ExternalOutput")
    idx_d = nc.dram_tensor("cand_idx", (B, NW), mybir.dt.uint16, kind="ExternalOutput")

    DR = mybir.MatmulPerfMode.DoubleRow
    MAX = mybir.AluOpType.max
    SUB = mybir.AluOpType.subtract

    with TileContext(nc) as tc:
        with (
            tc.tile_pool(name="const", bufs=1) as cpool,
            tc.tile_pool(name="scp", bufs=4) as scpool,
            tc.tile_pool(name="fp", bufs=2) as fpool,
            tc.tile_pool(name="psp", bufs=2, space="PSUM") as ppool,
        ):
            def load_qt(blk):
                t = cpool.tile([P, 2, 2, P], mybir.dt.float8e4, tag=f"qt{blk}", name=f"qt{blk}")
                nc.sync.dma_start(t, qt_d.ap()[blk])
                return t

            def load_xt(c, s):
                w = SLAB if s < NFULL else WPART
                t = cpool.tile([P, 2, w], mybir.dt.float8e4, tag=f"xt{c}_{s}", name=f"xt{c}_{s}")
                nc.sync.dma_start(t, xt_d.ap()[c][:, :, s * SLAB : s * SLAB + w])
                return t

            # first slab's inputs first so the PE can start ASAP
            qts = [load_qt(0)]
            xts = {(c, 0): load_xt(c, 0) for c in (0, 1)}
            qts += [load_qt(b) for b in range(1, BLK)]
            for s in range(1, NSLABS):
                for c in (0, 1):
                    xts[(c, s)] = load_xt(c, s)
            c_t = cpool.tile([P, NSLABS, GB, NWIN], mybir.dt.float16, tag="cwin", name="cwin")
            nc.sync.dma_start(c_t, c_d.ap())

            warm = cpool.tile([P, P], mybir.dt.float32, tag="warm", name="warm")
            nc.vector.memset(warm, 0.0)
            cv = cpool.tile([P, BLK, NW], mybir.dt.float16, tag="cv")
            ci = cpool.tile([P, BLK, NW], mybir.dt.uint16, tag="ci")

            # Warm-up: dummy matmuls with no DMA deps run during the initial
            # input-DMA wait so HAM un-throttles before real work.
            wps = ppool.tile([P, 4, NF], mybir.dt.float32, tag="ps", name="wps")
            for _ in range(24):
                nc.tensor.matmul(wps[:, 0, :P], warm, warm, start=True, stop=True)

            cs_tiles = {}
            drain_ctr = 0
            for s in range(NSLABS):
                w = SLAB if s < NFULL else WPART
                nch = w // NF
                nwin = w // RW
                sct = "scg" if s < NFULL else "scgp"
                for g in range(BLK // GB):
                    scg = scpool.tile([P, GB, nch, NF], mybir.dt.float16, tag=sct)
                    f1 = None
                    if s < NFULL:
                        f1 = fpool.tile([P, GB, 2, NF], mybir.dt.float16, tag="f1")
                    for b in range(GB):
                        blk = g * GB + b
                        ps = ppool.tile([P, 4, NF], mybir.dt.float32, tag="ps", name=f"ps{s}_{blk}")
                        for cpass in (0, 1):
                            for ch in range(nch):
                                nc.tensor.matmul(
                                    ps[:, ch, :],
                                    qts[blk][:, cpass],
                                    xts[(cpass, s)][:, :, ch * NF : (ch + 1) * NF],
                                    start=(cpass == 0),
                                    stop=(cpass == 1),
                                    perf_mode=DR,
                                )
                        # type-B block (last of each full-slab group): ACT
                        # drains only chunks 0-1; DVE computes its fold1 level
                        # directly against the still-in-PSUM chunks 2-3.
                        if s < NFULL and b == GB - 1:
                            nc.scalar.copy(scg[:, b, :2, :], ps[:, :2, :])
                            nc.vector.tensor_tensor(
                                out=f1[:, b],
                                in0=ps[:, 2:, :],
                                in1=scg[:, b, :2, :],
                                op=MAX,
                            )
                        else:
                            nc.scalar.copy(scg[:, b], ps[:, :nch, :])
                        drain_ctr += 1

                    # fold tree: pool columns mod (w//16) across the group
                    if s < NFULL:
                        nc.vector.tensor_tensor(
                            out=f1[:, : GB - 1],
                            in0=scg[:, : GB - 1, :2, :],
                            in1=scg[:, : GB - 1, 2:, :],
                            op=MAX,
                        )
                        f2 = fpool.tile([P, GB, NF], mybir.dt.float16, tag="f2")
                        nc.vector.tensor_tensor(
                            out=f2, in0=f1[:, :, 0, :], in1=f1[:, :, 1, :], op=MAX
                        )
                        prev, pw = f2, NF
                    else:
                        prev, pw = scg[:, :, 0, :], NF
                    lvl = 0
                    while pw > nwin:
                        pw //= 2
                        nxt = fpool.tile([P, GB, pw], mybir.dt.float16, tag=f"f{sct}{lvl}")
                        nc.vector.tensor_tensor(
                            out=nxt, in0=prev[:, :, :pw], in1=prev[:, :, pw:], op=MAX
                        )
                        prev = nxt
                        lvl += 1
                    if s < NFULL:
                        # c-subtract into the slab-pair extraction tile
                        if s % 2 == 0:
                            cs_tiles[g] = fpool.tile(
                                [P, GB, 2 * NWIN],
                                mybir.dt.float16,
                                tag=f"cs{g}",
                                name=f"cs{g}_{s}",
                            )
                        cs = cs_tiles[g]
                        nc.vector.tensor_tensor(
                            out=cs[:, :, (s % 2) * NWIN : (s % 2 + 1) * NWIN],
                            in0=prev,
                            in1=c_t[:, s],
                            op=SUB,
                        )
                        if s % 2 == 1:
                            pair = s // 2
                            for b in range(GB):
                                blk = g * GB + b
                                mv = cv[:, blk, pair * 8 : (pair + 1) * 8]
                                nc.vector.max(out=mv, in_=cs[:, b])
                                nc.vector.max_index(
                                    out=ci[:, blk, pair * 8 : (pair + 1) * 8],
                                    in_max=mv,
                                    in_values=cs[:, b],
                                )
                    else:
                        csp = fpool.tile([P, GB, NWINP], mybir.dt.float16, tag="csp")
                        nc.vector.tensor_tensor(
                            out=csp, in0=prev, in1=c_t[:, s, :, :NWINP], op=SUB
                        )
                        for b in range(GB):
                            blk = g * GB + b
                            mv = cv[:, blk, NPAIR * 8 : (NPAIR + 1) * 8]
                            nc.vector.max(out=mv, in_=csp[:, b])
                            nc.vector.max_index(
                                out=ci[:, blk, NPAIR * 8 : (NPAIR + 1) * 8],
                                in_max=mv,
                                in_values=csp[:, b],
                            )

            nc.sync.dma_start(vals_d.ap().rearrange("(blk p) j -> p blk j", p=P), cv)
            nc.sync.dma_start(idx_d.ap().rearrange("(blk p) j -> p blk j", p=P), ci)

    nc.compile()
    return nc


def _q8(a):
    return np.clip(a, -240.0, 240.0).astype(E4)


def _permutation():
    """sorted-rank r -> device column, for one core (NSH points).

    Window w = r//16 (16 norm-consecutive points); full-slab windows are
    dealt round-robin across the 6 full slabs; the remainder go to the
    partial slab. Returns dev_col[r]."""
    r = np.arange(NSH)
    wnd = r // RW
    j = r % RW
    full = wnd < NFWIN
    s = wnd % NFULL
    k = wnd // NFULL
    col_full = s * SLAB + j * NWIN + k
    pk = wnd - NFWIN
    col_part = NFULL * SLAB + j * NWINP + pk
    return np.where(full, col_full, col_part)


def _prepare_inputs(queries, X):
    queries = np.asarray(queries, np.float32)
    X = np.asarray(X, np.float32)

    q8 = _q8(2.0 * queries)  # [B, D]
    # qt[blk, p, c, i, m] = q8[blk*128+m, c*256+i*128+p]
    qt = np.ascontiguousarray(q8.reshape(BLK, P, 2, 2, P).transpose(0, 4, 2, 3, 1))

    dev_col = _permutation()  # [NSH]
    in_maps = []
    orig_maps = []
    for core in range(NCORES):
        sl = slice(core * NSH, (core + 1) * NSH)
        Xc = X[sl]
        x2 = (Xc.astype(np.float64) ** 2).sum(1)
        order = np.argsort(x2, kind="stable")  # ascending norm

        Xdev = np.zeros((NPAD, D), np.float32)
        Xdev[dev_col] = Xc[order]
        orig_of_col = np.full(NPAD, -1, np.int64)
        orig_of_col[dev_col] = core * NSH + order
        orig_maps.append(orig_of_col)

        x8 = _q8(Xdev)
        # xt[c, p, i, n] = x8[n, c*256+i*128+p]
        xt = np.ascontiguousarray(x8.reshape(NPAD, 2, 2, P).transpose(1, 3, 2, 0))

        # c[s, k] = min ||x||^2 of window (s, k); +30000 for empty windows
        cw = np.full((NSLABS, NWIN), 30000.0, np.float64)
        x2s = x2[order]
        wmin = np.minimum.reduceat(x2s, np.arange(0, NSH, RW))
        nwnd = wmin.shape[0]
        wi = np.arange(nwnd)
        full = wi < NFWIN
        cw[wi[full] % NFULL, wi[full] // NFULL] = wmin[full]
        cw[NFULL, wi[~full] - NFWIN] = wmin[~full]
        cwin = np.broadcast_to(
            cw.astype(np.float16)[None, :, None, :], (P, NSLABS, GB, NWIN)
        )
        in_maps.append({"qt": qt, "xt": xt, "cwin": np.ascontiguousarray(cwin)})
    return in_maps, orig_maps


def _run_device(queries, X, trace=False, trace_kwargs=None):
    global _prog
    if _prog is None:
        _prog = _build_program()
    in_maps, orig_maps = _prepare_inputs(queries, X)
    res = bass_utils.run_bass_kernel_spmd(
        _prog,
        in_maps,
        core_ids=list(range(NCORES)),
        trace=trace,
        **(trace_kwargs or {}),
    )
    res.orig_maps = orig_maps
    return res


def _merge(queries, X, Y, K, res):
    vals = np.stack([res.results[c]["cand_vals"] for c in range(NCORES)])  # [8,B,32]
    idxs = np.stack([res.results[c]["cand_idx"] for c in range(NCORES)]).astype(np.int64)
    orig = np.stack(res.orig_maps)  # [8, NPAD] original X row per device col, -1 pad

    # slot j: pair p=j//8 (<NPAIR: slabs 2p,2p+1, idx in [0,256)); else partial
    pair_of = np.arange(NW) // 8
    is_part = pair_of >= NPAIR

    av = vals.transpose(1, 0, 2).reshape(B, NCORES * NW).astype(np.float32)
    aw = idxs.transpose(1, 0, 2).reshape(B, NCORES * NW)
    pf = np.tile(pair_of, NCORES)[None, :]
    pp = np.tile(is_part, NCORES)[None, :]
    # decode window -> (first col, step) in core-local device columns
    slab = np.where(pp, NFULL, 2 * pf + (aw >= NWIN))
    wloc = np.where(pp, aw, aw % NWIN)
    col0 = slab * SLAB + wloc
    wstep = np.where(pp, NWINP, NWIN)
    core_of = np.repeat(np.arange(NCORES), NW)[None, :]

    K = int(K)
    sel = np.argpartition(-av, TOPW - 1, axis=1)[:, :TOPW]  # [B, TOPW]
    selc0 = np.take_along_axis(col0, sel, 1)
    selst = np.take_along_axis(np.broadcast_to(wstep, av.shape), sel, 1)
    selco = np.take_along_axis(np.broadcast_to(core_of, av.shape), sel, 1)
    cols = selc0[:, :, None] + selst[:, :, None] * np.arange(RW)[None, None, :]
    cols = cols.reshape(B, TOPW * RW)
    cores = np.repeat(selco, RW, axis=1)
    cand = orig[cores, cols]  # [B, TOPW*RW] original X row or -1
    invalid = cand < 0
    cand = np.where(invalid, 0, cand)

    qs = np.asarray(queries, np.float64)
    Xf = np.asarray(X, np.float64)
    CB = 64
    top = np.empty((B, K), np.int64)
    for i in range(0, B, CB):
        j = min(i + CB, B)
        Xc = Xf[cand[i:j].reshape(-1)].reshape(j - i, -1, D)
        d2 = ((Xc - qs[i:j, None, :]) ** 2).sum(-1)
        d2 += invalid[i:j] * 1e30
        order = np.argsort(d2, axis=1, kind="stable")[:, :K]
        top[i:j] = np.take_along_axis(cand[i:j], order, 1)

    labels = np.asarray(Y)[top].astype(np.float32)
    votes = labels.mean(1)
    out = np.zeros((B, 2), np.float32)
    out[:, 0] = votes
    return out


def kernel(queries, X, Y, K):
    res = _run_device(queries, X)
    return _merge(queries, X, Y, K, res)


# revision 14
# speedup vs baseline: 1.3332x; 1.3332x over previous
"""KNN classify kernel for TRN2 (8 NeuronCores).

Strategy: shard X over N (12500 points/core, padded to 12800). Scores are
computed as s[b,n] = 2*q.x with two fp8e4 DoubleRow matmuls (K_eff=256 each
over the 512 dims). The -||x||^2 term is handled structurally: the host
sorts each core's points by ||x||^2 and permutes columns so that each
pooling window (16 columns congruent mod 128 within a 2048-wide slab) holds
16 norm-consecutive points, dealt round-robin across slabs. PSUM tiles are
drained to SBUF fp16 (Scalar engine mostly, DVE every 8th), folded by a DVE
pairwise-max tree to 128 window maxima per slab, then a per-window constant
c[w] = min ||x||^2 (an fp16 input) is subtracted so window values
approximate max(2qx - x^2) with error bounded by the within-window norm
spread. MAX8/FIND_INDEX8 extract the top-8 windows per slab-pair; the host
expands the top windows (16 columns each), rescores exactly in fp64, takes
top-K and emits label votes.
"""

import sys

sys.path.insert(0, "/opt/trn_rl_repo")

import ml_dtypes
import numpy as np

import concourse.bacc as bacc
import concourse.mybir as mybir
from concourse import bass_utils
from concourse.tile import TileContext

B, D, N = 2048, 512, 100000
NCORES = 8
NSH = N // NCORES  # 12500
NPAD = 12800  # 25 * 512
P = 128
BLK = B // P  # 16 query blocks
NF = 512  # psum bank width (fp32 cols)
SLAB = 2048  # 4 psum banks
NFULL = 6  # full slabs; + 1 partial slab of 512
WPART = NPAD - NFULL * SLAB  # 512
NSLABS = NFULL + 1
RW = 16  # columns per pooled window
NWIN = SLAB // RW  # 128 windows per full slab
NWINP = WPART // RW  # 32 windows in partial slab
NFWIN = NFULL * NWIN  # 768 windows live in full slabs; the rest go to the partial
NPAIR = NFULL // 2  # slab pairs sharing one top-8 extraction
NW = (NPAIR + 1) * 8  # 32 candidates per (core, query)
GB = 4  # query-block group size for batched folds
DVE_DRAIN_EVERY = 8  # every Nth slab-drain goes to DVE instead of Scalar

E4 = ml_dtypes.float8_e4m3
TOPW = 40  # windows expanded+rescored on host per query

_prog = None


def _build_program():
    nc = bacc.Bacc("TRN2", target_bir_lowering=False, debug=False, num_devices=NCORES)
    qt_d = nc.dram_tensor("qt", (BLK, P, 2, 2, P), mybir.dt.float8e4, kind="ExternalInput")
    xt_d = nc.dram_tensor("xt", (2, P, 2, NPAD), mybir.dt.float8e4, kind="ExternalInput")
    c_d = nc.dram_tensor("cwin", (P, NSLABS, GB, NWIN), mybir.dt.float16, kind="ExternalInput")
    vals_d = nc.dram_tensor("cand_vals", (B, NW), mybir.dt.float16, kind="ExternalOutput")
    idx_d = nc.dram_tensor("cand_idx", (B, NW), mybir.dt.uint16, kind="ExternalOutput")

    DR = mybir.MatmulPerfMode.DoubleRow
    MAX = mybir.AluOpType.max
    SUB = mybir.AluOpType.subtract

    with TileContext(nc) as tc:
        with (
            tc.tile_pool(name="const", bufs=1) as cpool,
            tc.tile_pool(name="scp", bufs=4) as scpool,
            tc.tile_pool(name="fp", bufs=2) as fpool,
            tc.tile_pool(name="psp", bufs=2, space="PSUM") as ppool,
        ):
            def load_qt(blk):
                t = cpool.tile([P, 2, 2, P], mybir.dt.float8e4, tag=f"qt{blk}", name=f"qt{blk}")
                nc.sync.dma_start(t, qt_d.ap()[blk])
                return t

            def load_xt(c, s):
                w = SLAB if s < NFULL else WPART
                t = cpool.tile([P, 2, w], mybir.dt.float8e4, tag=f"xt{c}_{s}", name=f"xt{c}_{s}")
                nc.sync.dma_start(t, xt_d.ap()[c][:, :, s * SLAB : s * SLAB + w])
                return t

            # first slab's inputs first so the PE can start ASAP
            qts = [load_qt(0)]
            xts = {(c, 0): load_xt(c, 0) for c in (0, 1)}
            qts += [load_qt(b) for b in range(1, BLK)]
            for s in range(1, NSLABS):
                for c in (0, 1):
                    xts[(c, s)] = load_xt(c, s)
            c_t = cpool.tile([P, NSLABS, GB, NWIN], mybir.dt.float16, tag="cwin", name="cwin")
            nc.sync.dma_start(c_t, c_d.ap())

            warm = cpool.tile([P, P], mybir.dt.float32, tag="warm", name="warm")
            nc.vector.memset(warm, 0.0)
            cv = cpool.tile([P, BLK, NW], mybir.dt.float16, tag="cv")
            ci = cpool.tile([P, BLK, NW], mybir.dt.uint16, tag="ci")

            # Warm-up: dummy matmuls with no DMA deps run during the initial
            # input-DMA wait so HAM un-throttles before real work.
            wps = ppool.tile([P, 4, NF], mybir.dt.float32, tag="ps", name="wps")
            for _ in range(24):
                nc.tensor.matmul(wps[:, 0, :P], warm, warm, start=True, stop=True)

            cs_tiles = {}
            drain_ctr = 0
            for s in range(NSLABS):
                w = SLAB if s < NFULL else WPART
                nch = w // NF
                nwin = w // RW
                sct = "scg" if s < NFULL else "scgp"
                for g in range(BLK // GB):
                    scg = scpool.tile([P, GB, nch, NF], mybir.dt.float16, tag=sct)
                    for b in range(GB):
                        blk = g * GB + b
                        ps = ppool.tile([P, 4, NF], mybir.dt.float32, tag="ps", name=f"ps{s}_{blk}")
                        for cpass in (0, 1):
                            for ch in range(nch):
                                nc.tensor.matmul(
                                    ps[:, ch, :],
                                    qts[blk][:, cpass],
                                    xts[(cpass, s)][:, :, ch * NF : (ch + 1) * NF],
                                    start=(cpass == 0),
                                    stop=(cpass == 1),
                                    perf_mode=DR,
                                )
                        dst = scg[:, b]
                        src = ps[:, :nch, :]
                        if (
                            s < NFULL
                            and drain_ctr % DVE_DRAIN_EVERY == DVE_DRAIN_EVERY - 1
                        ):
                            nc.vector.tensor_copy(out=dst, in_=src)
                        else:
                            nc.scalar.copy(dst, src)
                        drain_ctr += 1

                    # fold tree: pool columns mod (w//16) across the group
                    if s < NFULL:
                        f1 = fpool.tile([P, GB, 2, NF], mybir.dt.float16, tag="f1")
                        nc.vector.tensor_tensor(
                            out=f1, in0=scg[:, :, :2, :], in1=scg[:, :, 2:, :], op=MAX
                        )
                        f2 = fpool.tile([P, GB, NF], mybir.dt.float16, tag="f2")
                        nc.vector.tensor_tensor(
                            out=f2, in0=f1[:, :, 0, :], in1=f1[:, :, 1, :], op=MAX
                        )
                        prev, pw = f2, NF
                    else:
                        prev, pw = scg[:, :, 0, :], NF
                    lvl = 0
                    while pw > nwin:
                        pw //= 2
                        nxt = fpool.tile([P, GB, pw], mybir.dt.float16, tag=f"f{sct}{lvl}")
                        nc.vector.tensor_tensor(
                            out=nxt, in0=prev[:, :, :pw], in1=prev[:, :, pw:], op=MAX
                        )
                        prev = nxt
                        lvl += 1
                    if s < NFULL:
                        # c-subtract into the slab-pair extraction tile
                        if s % 2 == 0:
                            cs_tiles[g] = fpool.tile(
                                [P, GB, 2 * NWIN],
                                mybir.dt.float16,
                                tag=f"cs{g}",
                                name=f"cs{g}_{s}",
                            )
                        cs = cs_tiles[g]
                        nc.vector.tensor_tensor(
                            out=cs[:, :, (s % 2) * NWIN : (s % 2 + 1) * NWIN],
                            in0=prev,
                            in1=c_t[:, s],
                            op=SUB,
                        )
                        if s % 2 == 1:
                            pair = s // 2
                            for b in range(GB):
                                blk = g * GB + b
                                mv = cv[:, blk, pair * 8 : (pair + 1) * 8]
                                nc.vector.max(out=mv, in_=cs[:, b])
                                nc.vector.max_index(
                                    out=ci[:, blk, pair * 8 : (pair + 1) * 8],
                                    in_max=mv,
                                    in_values=cs[:, b],
                                )
                    else:
                        csp = fpool.tile([P, GB, NWINP], mybir.dt.float16, tag="csp")
                        nc.vector.tensor_tensor(
                            out=csp, in0=prev, in1=c_t[:, s, :, :NWINP], op=SUB
                        )
                        for b in range(GB):
                            blk = g * GB + b
                            mv = cv[:, blk, NPAIR * 8 : (NPAIR + 1) * 8]
                            nc.vector.max(out=mv, in_=csp[:, b])
                            nc.vector.max_index(
                                out=ci[:, blk, NPAIR * 8 : (NPAIR + 1) * 8],
                                in_max=mv,
                                in_values=csp[:, b],
                            )

            nc.sync.dma_start(vals_d.ap().rearrange("(blk p) j -> p blk j", p=P), cv)
            nc.sync.dma_start(idx_d.ap().rearrange("(blk p) j -> p blk j", p=P), ci)

    nc.compile()
    return nc


def _q8(a):
    return np.clip(a, -240.0, 240.0).astype(E4)


def _permutation():
    """sorted-rank r -> device column, for one core (NSH points).

    Window w = r//16 (16 norm-consecutive points); full-slab windows are
    dealt round-robin across the 6 full slabs; the remainder go to the
    partial slab. Returns dev_col[r]."""
    r = np.arange(NSH)
    wnd = r // RW
    j = r % RW
    full = wnd < NFWIN
    s = wnd % NFULL
    k = wnd // NFULL
    col_full = s * SLAB + j * NWIN + k
    pk = wnd - NFWIN
    col_part = NFULL * SLAB + j * NWINP + pk
    return np.where(full, col_full, col_part)


def _prepare_inputs(queries, X):
    queries = np.asarray(queries, np.float32)
    X = np.asarray(X, np.float32)

    q8 = _q8(2.0 * queries)  # [B, D]
    # qt[blk, p, c, i, m] = q8[blk*128+m, c*256+i*128+p]
    qt = np.ascontiguousarray(q8.reshape(BLK, P, 2, 2, P).transpose(0, 4, 2, 3, 1))

    dev_col = _permutation()  # [NSH]
    in_maps = []
    orig_maps = []
    for core in range(NCORES):
        sl = slice(core * NSH, (core + 1) * NSH)
        Xc = X[sl]
        x2 = (Xc.astype(np.float64) ** 2).sum(1)
        order = np.argsort(x2, kind="stable")  # ascending norm

        Xdev = np.zeros((NPAD, D), np.float32)
        Xdev[dev_col] = Xc[order]
        orig_of_col = np.full(NPAD, -1, np.int64)
        orig_of_col[dev_col] = core * NSH + order
        orig_maps.append(orig_of_col)

        x8 = _q8(Xdev)
        # xt[c, p, i, n] = x8[n, c*256+i*128+p]
        xt = np.ascontiguousarray(x8.reshape(NPAD, 2, 2, P).transpose(1, 3, 2, 0))

        # c[s, k] = min ||x||^2 of window (s, k); +30000 for empty windows
        cw = np.full((NSLABS, NWIN), 30000.0, np.float64)
        x2s = x2[order]
        wmin = np.minimum.reduceat(x2s, np.arange(0, NSH, RW))
        nwnd = wmin.shape[0]
        wi = np.arange(nwnd)
        full = wi < NFWIN
        cw[wi[full] % NFULL, wi[full] // NFULL] = wmin[full]
        cw[NFULL, wi[~full] - NFWIN] = wmin[~full]
        cwin = np.broadcast_to(
            cw.astype(np.float16)[None, :, None, :], (P, NSLABS, GB, NWIN)
        )
        in_maps.append({"qt": qt, "xt": xt, "cwin": np.ascontiguousarray(cwin)})
    return in_maps, orig_maps


def _run_device(queries, X, trace=False, trace_kwargs=None):
    global _prog
    if _prog is None:
        _prog = _build_program()
    in_maps, orig_maps = _prepare_inputs(queries, X)
    res = bass_utils.run_bass_kernel_spmd(
        _prog,
        in_maps,
        core_ids=list(range(NCORES)),
        trace=trace,
        **(trace_kwargs or {}),
    )
    res.orig_maps = orig_maps
    return res


def _merge(queries, X, Y, K, res):
    vals = np.stack([res.results[c]["cand_vals"] for c in range(NCORES)])  # [8,B,32]
    idxs = np.stack([res.results[c]["cand_idx"] for c in range(NCORES)]).astype(np.int64)
    orig = np.stack(res.orig_maps)  # [8, NPAD] original X row per device col, -1 pad

    # slot j: pair p=j//8 (<NPAIR: slabs 2p,2p+1, idx in [0,256)); else partial
    pair_of = np.arange(NW) // 8
    is_part = pair_of >= NPAIR

    av = vals.transpose(1, 0, 2).reshape(B, NCORES * NW).astype(np.float32)
    aw = idxs.transpose(1, 0, 2).reshape(B, NCORES * NW)
    pf = np.tile(pair_of, NCORES)[None, :]
    pp = np.tile(is_part, NCORES)[None, :]
    # decode window -> (first col, step) in core-local device columns
    slab = np.where(pp, NFULL, 2 * pf + (aw >= NWIN))
    wloc = np.where(pp, aw, aw % NWIN)
    col0 = slab * SLAB + wloc
    wstep = np.where(pp, NWINP, NWIN)
    core_of = np.repeat(np.arange(NCORES), NW)[None, :]

    K = int(K)
    sel = np.argpartition(-av, TOPW - 1, axis=1)[:, :TOPW]  # [B, TOPW]
    selc0 = np.take_along_axis(col0, sel, 1)
    selst = np.take_along_axis(np.broadcast_to(wstep, av.shape), sel, 1)
    selco = np.take_along_axis(np.broadcast_to(core_of, av.shape), sel, 1)
    cols = selc0[:, :, None] + selst[:, :, None] * np.arange(RW)[None, None, :]
    cols = cols.reshape(B, TOPW * RW)
    cores = np.repeat(selco, RW, axis=1)
    cand = orig[cores, cols]  # [B, TOPW*RW] original X row or -1
    invalid = cand < 0
    cand = np.where(invalid, 0, cand)

    qs = np.asarray(queries, np.float64)
    Xf = np.asarray(X, np.float64)
    CB = 64
    top = np.empty((B, K), np.int64)
    for i in range(0, B, CB):
        j = min(i + CB, B)
        Xc = Xf[cand[i:j].reshape(-1)].reshape(j - i, -1, D)
        d2 = ((Xc - qs[i:j, None, :]) ** 2).sum(-1)
        d2 += invalid[i:j] * 1e30
        order = np.argsort(d2, axis=1, kind="stable")[:, :K]
        top[i:j] = np.take_along_axis(cand[i:j], order, 1)

    labels = np.asarray(Y)[top].astype(np.float32)
    votes = labels.mean(1)
    out = np.zeros((B, 2), np.float32)
    out[:, 0] = votes
    return out


def kernel(queries, X, Y, K):
    res = _run_device(queries, X)
    return _merge(queries, X, Y, K, res)
